# revision 13
# baseline (speedup 1.0000x reference)
"""Trainium2 Bass kernel for nn_Encoder_86852828659979 (8-core SPMD).

Sharding (8 NeuronCores):
  - Attention: head-parallel. Core c owns head c: computes qT/kT/v for its
    head from replicated x^T, scoresT = (q@k^T)^T in [t, s] layout so the
    softmax reduction over t is a ones-matmul on the PE, z^T = v^T @ p^T,
    then its partial of the output projection z_h @ Wo_h.
  - The s dimension is processed in 4 chunks of 512 columns, permuted so
    each chunk's ReduceScatter hands every core a contiguous 64-row piece
    of its 256 target rows. The 4 RS collectives overlap attention compute.
  - Post-RS everything is sequence-parallel: each core does x+attn residual,
    LN1, the 4-layer FFN and LN2 on its own 256 rows (full FFN weights
    streamed from HBM as moving operands; activations stay transposed
    [feature, seq] so no transposes are needed between layers).
  - Host concatenates the 8 [256, 512] output shards.

All matmuls run in float32r (full PE rate at N>=256, ~1e-4 relative
precision). PSUM accumulation is fp32.
"""

import math

import numpy as np

import concourse.bacc as bacc
import concourse.mybir as mybir
import concourse.tile as tile
from concourse import bass_utils
from concourse.masks import make_identity

S, D, H, HID = 2048, 512, 8, 2048
P = 128
NCORE = 8
SC = S // NCORE          # 256 output rows per core
NCH = 4                  # attention s' chunks
CH = S // NCH            # 512 columns per chunk
EPS = 1e-5
F32 = mybir.dt.float32
F32R = mybir.dt.float32r
AF = mybir.ActivationFunctionType
ALU = mybir.AluOpType
AX = mybir.AxisListType

# bias_pack column layout ([128, 56] f32): col j holds slice [j*128:(j+1)*128]
BQ_COL, BK_COL, B1_COL, B2_COL, B3_COL = 0, 4, 8, 24, 40
# row_pack rows ([7, 512] f32, broadcast to all partitions)
BV_R, BO_R, B4_R, G1_R, BE1_R, G2_R, BE2_R = range(7)

_CACHE: dict = {}


def _layer_norm(nc, pool, stat, t, g_bc, b_bc, eps_tile, out_dtype=F32):
    """LN over the free dim (512) of a [128, 512] tile. Returns a new tile."""
    mean = stat.tile([P, 1], F32, tag="stat")
    nc.vector.tensor_reduce(mean[:], t[:], axis=AX.X, op=ALU.add)
    nc.vector.tensor_scalar_mul(mean[:], mean[:], 1.0 / D)
    zc = pool.tile([P, D], F32, tag="lnz")
    nc.vector.tensor_scalar(zc[:], t[:], mean[:], None, op0=ALU.subtract)
    sq = pool.tile([P, D], F32, tag="lnsq")
    var = stat.tile([P, 1], F32, tag="stat")
    nc.scalar.activation(sq[:], zc[:], AF.Square, accum_out=var[:])
    std = stat.tile([P, 1], F32, tag="stat")
    nc.scalar.activation(std[:], var[:], AF.Sqrt, bias=eps_tile[:], scale=1.0 / D)
    rstd = stat.tile([P, 1], F32, tag="stat")
    nc.vector.reciprocal(rstd[:], std[:])
    out = pool.tile([P, D], out_dtype, tag="lnout")
    nc.vector.tensor_scalar(out[:], zc[:], rstd[:], None, op0=ALU.mult)
    nc.vector.tensor_mul(out[:], out[:], g_bc)
    nc.vector.tensor_add(out[:], out[:], b_bc)
    return out


def _build():
    nc = bacc.Bacc("TRN2", target_bir_lowering=False, debug=False, num_devices=NCORE)

    xt_d = nc.dram_tensor("xt", [D, S], F32R, kind="ExternalInput")
    wq_d = nc.dram_tensor("wq", [D, D], F32R, kind="ExternalInput")
    wk_d = nc.dram_tensor("wk", [D, D], F32R, kind="ExternalInput")
    wv_d = nc.dram_tensor("wv", [D, D], F32R, kind="ExternalInput")
    wo_d = nc.dram_tensor("wo", [D, D], F32R, kind="ExternalInput")
    w1_d = nc.dram_tensor("w1", [D, HID], F32R, kind="ExternalInput")
    w2_d = nc.dram_tensor("w2", [HID, HID], F32R, kind="ExternalInput")
    w3_d = nc.dram_tensor("w3", [HID, HID], F32R, kind="ExternalInput")
    w4_d = nc.dram_tensor("w4", [HID, D], F32R, kind="ExternalInput")
    bias_d = nc.dram_tensor("biasp", [P, 56], F32, kind="ExternalInput")
    rowv_d = nc.dram_tensor("rowv", [7, D], F32, kind="ExternalInput")
    xres_d = nc.dram_tensor("xres", [SC, D], F32, kind="ExternalInput")
    out_d = nc.dram_tensor("out", [SC, D], F32, kind="ExternalOutput")

    rg = [list(range(NCORE))]

    with tile.TileContext(nc) as tc:
        import contextlib

        with contextlib.ExitStack() as ctx:
            const = ctx.enter_context(tc.tile_pool(name="const", bufs=1))
            stat = ctx.enter_context(tc.tile_pool(name="stat", bufs=8))
            dram = ctx.enter_context(tc.tile_pool(name="dram", bufs=4, space="DRAM"))
            ps = ctx.enter_context(tc.tile_pool(name="ps", bufs=8, space="PSUM"))

            bias_sb = const.tile([P, 56], F32)
            nc.sync.dma_start(bias_sb[:], bias_d[:, :])
            row_sb = const.tile([P, 7 * D], F32)
            rowv_bc = tile.bass.AP(
                tensor=rowv_d.ap().tensor,
                offset=rowv_d.ap().offset,
                ap=[[0, P], [1, 7 * D]],
            )
            nc.sync.dma_start(row_sb[:], rowv_bc)

            def row(i):
                return row_sb[:, i * D:(i + 1) * D]

            ones_f = const.tile([P, P], F32)
            nc.vector.memset(ones_f[:], 1.0)
            ones = const.tile([P, P], F32R)
            nc.scalar.copy(ones[:], ones_f[:])
            ident = const.tile([P, P], F32)
            make_identity(nc, ident[:])
            eps_t = const.tile([P, 1], F32)
            nc.vector.memset(eps_t[:], EPS)

            # ---------------- phase 1+2: attention ----------------
            zp_bs = []
            with (
                tc.tile_pool(name="wo_p", bufs=1) as wo_p,
                tc.tile_pool(name="qt", bufs=1) as qt_p,
                tc.tile_pool(name="kt", bufs=1) as kt_p,
                tc.tile_pool(name="v", bufs=1) as v_p,
            ):
                with (
                    tc.tile_pool(name="xt", bufs=1) as xt_p,
                    tc.tile_pool(name="qkv_w", bufs=1) as qkv_w,
                ):
                    xt = []
                    for i in range(4):
                        t = xt_p.tile([P, S], F32R, tag=f"xt{i}")
                        nc.sync.dma_start(t[:], xt_d[i * P:(i + 1) * P, :])
                        xt.append(t)

                    def load_w(dram_t, name, pool):
                        ts = []
                        for i in range(4):
                            t = pool.tile([P, D], F32R, tag=f"{name}{i}")
                            nc.sync.dma_start(t[:], dram_t[i * P:(i + 1) * P, :])
                            ts.append(t)
                        return ts

                    wq_t = load_w(wq_d, "wq", qkv_w)
                    wk_t = load_w(wk_d, "wk", qkv_w)
                    wv_t = load_w(wv_d, "wv", qkv_w)
                    wo_t = load_w(wo_d, "wo", wo_p)

                    qt, kt = [], []
                    for dst, w_t, bcol in ((qt, wq_t, BQ_COL), (kt, wk_t, BK_COL)):
                        pool = qt_p if bcol == BQ_COL else kt_p
                        for m in range(4):
                            t = pool.tile([P, S], F32R, tag=f"t{m}")
                            dst.append(t)
                            for n in range(4):
                                pt = ps.tile([P, CH], F32, tag="ps")
                                for kk in range(4):
                                    nc.tensor.matmul(
                                        pt[:],
                                        w_t[kk][:, m * P:(m + 1) * P],
                                        xt[kk][:, n * CH:(n + 1) * CH],
                                        start=(kk == 0), stop=(kk == 3),
                                    )
                                nc.scalar.activation(
                                    t[:, n * CH:(n + 1) * CH], pt[:], AF.Identity,
                                    bias=bias_sb[:, bcol + m:bcol + m + 1],
                                )

                    vt = []
                    for m in range(16):
                        t = v_p.tile([P, D], F32R, tag=f"v{m}")
                        vt.append(t)
                        pt = ps.tile([P, D], F32, tag="ps")
                        for kk in range(4):
                            nc.tensor.matmul(
                                pt[:],
                                xt[kk][:, m * P:(m + 1) * P],
                                wv_t[kk][:],
                                start=(kk == 0), stop=(kk == 3),
                            )
                        nc.vector.tensor_tensor(t[:], pt[:], row(BV_R), op=ALU.add)

                attn_ctx = contextlib.ExitStack()
                expt_p = attn_ctx.enter_context(tc.tile_pool(name="expt", bufs=17))
                zt_p = attn_ctx.enter_context(tc.tile_pool(name="zt", bufs=5))
                zosb_p = attn_ctx.enter_context(tc.tile_pool(name="zosb", bufs=2))
                recip_p = attn_ctx.enter_context(tc.tile_pool(name="recip", bufs=2))
                inv_sqrt_d = 1.0 / math.sqrt(D)
                for k in range(NCH):
                    ps_sum = ps.tile([P, CH], F32, tag="ps")
                    expt = []
                    for m in range(16):
                        pt = ps.tile([P, CH], F32, tag="ps")
                        for kk in range(4):
                            nc.tensor.matmul(
                                pt[:],
                                kt[kk][:, m * P:(m + 1) * P],
                                qt[kk][:, k * CH:(k + 1) * CH],
                                start=(kk == 0), stop=(kk == 3),
                            )
                        et = expt_p.tile([P, CH], F32R, tag="expt")
                        nc.scalar.activation(et[:], pt[:], AF.Exp)
                        expt.append(et)
                        nc.tensor.matmul(
                            ps_sum[:], ones[:], et[:],
                            start=(m == 0), stop=(m == 15),
                        )
                    recip = recip_p.tile([P, CH], F32, tag="recip")
                    nc.vector.reciprocal(recip[:], ps_sum[:])
                    nc.vector.tensor_scalar_mul(recip[:], recip[:], inv_sqrt_d)

                    zt = []
                    for e in range(4):
                        pt = ps.tile([P, CH], F32, tag="ps")
                        for m in range(16):
                            nc.tensor.matmul(
                                pt[:],
                                vt[m][:, e * P:(e + 1) * P],
                                expt[m][:],
                                start=(m == 0), stop=(m == 15),
                            )
                        zte = zt_p.tile([P, CH], F32R, tag="zt")
                        nc.vector.tensor_tensor(zte[:], pt[:], recip[:], op=ALU.mult)
                        zt.append(zte)

                    zo_b = dram.tile([CH, D], F32, tag="zob")
                    for m in range(4):
                        pt = ps.tile([P, D], F32, tag="ps")
                        for e in range(4):
                            nc.tensor.matmul(
                                pt[:],
                                zt[e][:, m * P:(m + 1) * P],
                                wo_t[e][:],
                                start=(e == 0), stop=(e == 3),
                            )
                        zo_sb = zosb_p.tile([P, D], F32, tag="zosb")
                        nc.scalar.copy(zo_sb[:], pt[:])
                        nc.sync.dma_start(zo_b[m * P:(m + 1) * P, :], zo_sb[:])
                    zp_b = dram.tile([CH // NCORE, D], F32, tag="zpb")
                    nc.gpsimd.collective_compute(
                        "ReduceScatter", ALU.add, replica_groups=rg,
                        ins=[zo_b.opt()], outs=[zp_b.opt()],
                    )
                    zp_bs.append(zp_b)
                attn_ctx.close()

            # ---------------- phase 3: residual + LN1 ----------------
            ln_p = ctx.enter_context(tc.tile_pool(name="ln", bufs=2))
            zres_p = ctx.enter_context(tc.tile_pool(name="zres", bufs=1))
            z_sb = []
            for si in range(2):
                zin = ln_p.tile([P, D], F32, tag="zin")
                for kk in (2 * si, 2 * si + 1):
                    nc.sync.dma_start(
                        zin[(kk % 2) * 64:(kk % 2) * 64 + 64, :], zp_bs[kk][:]
                    )
                xr = ln_p.tile([P, D], F32, tag="xr")
                nc.sync.dma_start(xr[:], xres_d[si * P:(si + 1) * P, :])
                nc.vector.tensor_add(zin[:], zin[:], xr[:])
                nc.vector.tensor_add(zin[:], zin[:], row(BO_R))
                zo_ = _layer_norm(nc, ln_p, stat, zin, row(G1_R), row(BE1_R), eps_t)
                zres = zres_p.tile([P, D], F32, tag=f"zres{si}")
                nc.vector.tensor_copy(zres[:], zo_[:])
                z_sb.append(zres)

            # ---------------- phase 4: transpose z -> [d, s] ----------------
            ztf_p = ctx.enter_context(tc.tile_pool(name="ztf", bufs=1))
            ztf = [
                ztf_p.tile([P, SC], F32R, tag=f"ztf{j}", name=f"ztf{j}")
                for j in range(4)
            ]
            for si in range(2):
                for j in range(4):
                    pt = ps.tile([P, P], F32, tag="ps")
                    nc.tensor.transpose(
                        pt[:], z_sb[si][:, j * P:(j + 1) * P], ident[:]
                    )
                    nc.scalar.copy(ztf[j][:, si * P:(si + 1) * P], pt[:])

            # ---------------- phase 5: FFN ----------------
            with (
                tc.tile_pool(name="wstream", bufs=4) as wst_p,
                tc.tile_pool(name="h1t", bufs=1) as h1_p,
                tc.tile_pool(name="h2t", bufs=1) as h2_p,
                tc.tile_pool(name="h3t", bufs=1) as h3_p,
                tc.tile_pool(name="w4p", bufs=1) as w4_p,
            ):
                with tc.tile_pool(name="w1p", bufs=1) as w1_p:
                    w1_t = []
                    for i in range(4):
                        t = w1_p.tile([P, HID], F32R, tag=f"w1{i}")
                        nc.sync.dma_start(t[:], w1_d[i * P:(i + 1) * P, :])
                        w1_t.append(t)

                    h1t = []
                    for m in range(16):
                        pt = ps.tile([P, SC], F32, tag="ps")
                        for kk in range(4):
                            nc.tensor.matmul(
                                pt[:],
                                w1_t[kk][:, m * P:(m + 1) * P],
                                ztf[kk][:],
                                start=(kk == 0), stop=(kk == 3),
                            )
                        t = h1_p.tile([P, SC], F32R, tag=f"h1{m}")
                        nc.scalar.activation(
                            t[:], pt[:], AF.Relu,
                            bias=bias_sb[:, B1_COL + m:B1_COL + m + 1],
                        )
                        h1t.append(t)

                def big_layer(w_d, h_in, h_pool, hname, bcol):
                    h_out = []
                    for mg in range(2):
                        pss = [
                            ps.tile([P, SC], F32, tag="ps", name=f"ps_{hname}{mg}_{m}")
                            for m in range(8)
                        ]
                        for kk in range(16):
                            wh = wst_p.tile([P, 1024], F32R, tag="wstream")
                            nc.sync.dma_start(
                                wh[:],
                                w_d[kk * P:(kk + 1) * P, mg * 1024:(mg + 1) * 1024],
                            )
                            for m in range(8):
                                nc.tensor.matmul(
                                    pss[m][:],
                                    wh[:, m * P:(m + 1) * P],
                                    h_in[kk][:],
                                    start=(kk == 0), stop=(kk == 15),
                                )
                        for m in range(8):
                            idx = mg * 8 + m
                            t = h_pool.tile([P, SC], F32R, tag=f"{hname}{idx}")
                            nc.scalar.activation(
                                t[:], pss[m][:], AF.Relu,
                                bias=bias_sb[:, bcol + idx:bcol + idx + 1],
                            )
                            h_out.append(t)
                    return h_out

                h2t = big_layer(w2_d, h1t, h2_p, "h2", B2_COL)
                h3t = big_layer(w3_d, h2t, h3_p, "h3", B3_COL)

                w4_t = []
                for i in range(16):
                    t = w4_p.tile([P, D], F32R, tag=f"w4{i}")
                    nc.sync.dma_start(t[:], w4_d[i * P:(i + 1) * P, :])
                    w4_t.append(t)

                for m in range(2):
                    pt = ps.tile([P, D], F32, tag="ps")
                    for kk in range(16):
                        nc.tensor.matmul(
                            pt[:],
                            h3t[kk][:, m * P:(m + 1) * P],
                            w4_t[kk][:],
                            start=(kk == 0), stop=(kk == 15),
                        )
                    u = ln_p.tile([P, D], F32, tag="u")
                    nc.vector.tensor_tensor(u[:], pt[:], row(B4_R), op=ALU.add)
                    nc.vector.tensor_add(u[:], u[:], z_sb[m][:])
                    o = _layer_norm(nc, ln_p, stat, u, row(G2_R), row(BE2_R), eps_t)
                    nc.sync.dma_start(out_d[m * P:(m + 1) * P, :], o[:])

    nc.compile()
    return nc


def _prep_inputs(inputs):
    f = lambda a: np.ascontiguousarray(np.asarray(a), dtype=np.float32)
    x = f(inputs["x"])
    xt = np.ascontiguousarray(x.T)                       # [D, S]
    # s' permutation: s = c*256 + k*64 + j  ->  s' = k*512 + c*64 + j
    xt_perm = np.ascontiguousarray(
        xt.reshape(D, NCORE, NCH, 64).transpose(0, 2, 1, 3).reshape(D, S)
    )
    Wq, Wk, Wv = f(inputs["Wq"]), f(inputs["Wk"]), f(inputs["Wv"])
    bq, bk, bv = f(inputs["bq"]), f(inputs["bk"]), f(inputs["bv"])
    Wo, bo = f(inputs["Wo"]), f(inputs["bo"])
    W1, W2, W3, W4 = f(inputs["W1"]), f(inputs["W2"]), f(inputs["W3"]), f(inputs["W4"])
    b1, b2, b3, b4 = f(inputs["b1"]), f(inputs["b2"]), f(inputs["b3"]), f(inputs["b4"])
    g1, be1 = f(inputs["ln1_g"]), f(inputs["ln1_b"])
    g2, be2 = f(inputs["ln2_g"]), f(inputs["ln2_b"])

    in_maps = []
    for c in range(NCORE):
        cols = (
            [bq[c][i * P:(i + 1) * P] for i in range(4)]
            + [bk[c][i * P:(i + 1) * P] for i in range(4)]
            + [b1[i * P:(i + 1) * P] for i in range(16)]
            + [b2[i * P:(i + 1) * P] for i in range(16)]
            + [b3[i * P:(i + 1) * P] for i in range(16)]
        )
        biasp = np.ascontiguousarray(np.stack(cols, axis=1))
        rowv = np.ascontiguousarray(
            np.stack([bv[c], bo, b4, g1, be1, g2, be2], axis=0)
        )
        in_maps.append({
            "xt": xt_perm,
            "wq": Wq[c], "wk": Wk[c], "wv": Wv[c],
            "wo": np.ascontiguousarray(Wo[c * D:(c + 1) * D, :]),
            "w1": W1, "w2": W2, "w3": W3, "w4": W4,
            "biasp": biasp, "rowv": rowv,
            "xres": np.ascontiguousarray(x[c * SC:(c + 1) * SC, :]),
        })
    return in_maps


def kernel(**inputs) -> np.ndarray:
    if "nc" not in _CACHE:
        _CACHE["nc"] = _build()
    nc = _CACHE["nc"]
    in_maps = _prep_inputs(inputs)
    r = bass_utils.run_bass_kernel_spmd(nc, in_maps, core_ids=list(range(NCORE)))
    return np.concatenate([r.results[c]["out"] for c in range(NCORE)], axis=0)
